# revision 16
# baseline (speedup 1.0000x reference)
"""Hadamard transform kernel for Trainium2 (8 NeuronCores, SPMD data-parallel).

Computes y = (x @ H^T) / sqrt(D), padded with a zero imaginary plane ->
[B, S, D, 2], for x [4, 4096, 1024] fp32 and H the 1024-point Hadamard
matrix (H[i,j] = (-1)^popcount(i&j), symmetric, Kronecker-structured).

Precision/layout choices (all inside kernel(), tolerance is 2e-2):
  - x is rounded to bf16 and pre-transposed per 128-row tile on the host
    during sharding (pure layout + the same rounding the on-chip pipeline
    would apply): halves load traffic and removes all PE transposes.
  - The device writes the real plane in bf16 (host upcasts to fp32 and
    interleaves the zero imaginary plane): halves store traffic.
  Measured end-to-end relative error ~3e-3.

Per-core traffic: 4 MiB in + 4 MiB out + 0.13 MiB weights (~24 us at the
360 GB/s DMA roofline); every engine stage fits under the per-tile DMA
budget, so the kernel is DMA-bound.

Math (shard of 2048 rows, 16 row-tiles of 128):
  H_1024 = H_4 (x) H_256  under d = a*256 + b, f = c*256 + e, with
  H_256[e, j*128+b'] = H2[e8, j] * H128[e_lo, b'] (e = e8*128 + e_lo).
  Stage 1 (PE, bf16): per quarter a in 0..4, 2 accumulating matmuls
    z_a += xt[:, (2a+j)*128:...]^T @ W2[:, j*256:(j+1)*256], where
    W2[b', j*256 + e8*128 + e_lo] = H2[e8,j] * H128[e_lo,b'] / 32
    (host-precomputed, exact +-2^-5 entries, bf16).
  Stage 2 (H4 butterfly over a, 256 cols/op, bf16 intermediates):
    stage z0,z1 -> SBUF (ACT), then dist-2: w0=z0+z2 (ACT), w1=z1+z3,
    w2=z0-z2, w3=z1-z3 (DVE); dist-1: y0=w0+w1, y1=w0-w1 (DVE, 4x bf16),
    y2=w2+w3, y3=w2-w3 (Pool). Quarter-stores ship as each y_c lands.
  Startup: all 16 loads queued on SP up front; W rides the ACT queue; a
  burst of dummy matmuls ramps the PE p-state during the first loads.
"""

import numpy as np
from contextlib import ExitStack

import concourse.bass as bass
import concourse.tile as tile
from concourse import bacc, bass_utils, mybir

N_CORES = 8
B, S, D = 4, 4096, 1024
ROWS = B * S                 # 16384
SHARD = ROWS // N_CORES      # 2048
NT = SHARD // 128            # 16 tiles of 128 rows per core
F32 = mybir.dt.float32
BF16 = mybir.dt.bfloat16

_cache = {}

CFG = {
    "xin_bufs": 16,
    "out_bufs": 4,
    "zs_bufs": 3,
    "w_bufs": 3,
    "z_bufs": 2,
    "warmup": 10,
}


def _build_nc(cfg=None):
    cfg = {**CFG, **(cfg or {})}
    nc = bacc.Bacc("TRN2", target_bir_lowering=False, debug=False)
    # xt: per tile t, xt[t*128+b', g*128+n] = x[t*128+n, g*128+b'] (bf16)
    xt_d = nc.dram_tensor("xt", [SHARD, D], BF16, kind="ExternalInput").ap()
    w_d = nc.dram_tensor("w", [128, 512], BF16, kind="ExternalInput").ap()
    o_d = nc.dram_tensor("out", [SHARD, D], BF16, kind="ExternalOutput").ap()

    with tile.TileContext(nc) as tc, ExitStack() as ctx:
        const_pool = ctx.enter_context(tc.tile_pool(name="const", bufs=1))
        xin_pool = ctx.enter_context(tc.tile_pool(name="xin", bufs=cfg["xin_bufs"]))
        out_pool = ctx.enter_context(tc.tile_pool(name="outp", bufs=cfg["out_bufs"]))
        zs_pool = ctx.enter_context(tc.tile_pool(name="zs", bufs=cfg["zs_bufs"]))
        wb_pool = ctx.enter_context(tc.tile_pool(name="wb", bufs=cfg["w_bufs"]))
        ps_z = [
            ctx.enter_context(
                tc.tile_pool(name=f"ps_z{a}", bufs=cfg["z_bufs"], space="PSUM"))
            for a in range(4)
        ]

        # All 16 xt loads queued on SP up front; W rides the ACT queue.
        xt_tiles = []
        for it in range(NT):
            xt_sb = xin_pool.tile([128, D], BF16, tag="xt")
            nc.sync.dma_start(xt_sb[:], xt_d[it * 128:(it + 1) * 128, :])
            xt_tiles.append(xt_sb)

        W_sb = const_pool.tile([128, 512], BF16, tag="W")
        nc.scalar.dma_start(W_sb[:], w_d[:])

        # PE p-state warmup: dummy matmuls on a zeroed tile while the first
        # loads are in flight (reusing the z0 PSUM pool).
        Zb_sb = const_pool.tile([128, 256], BF16, tag="Zb")
        nc.vector.memset(Zb_sb[:], 0.0)
        for _ in range(cfg["warmup"]):
            warm_ps = ps_z[0].tile([128, 256], F32, tag="z0")
            nc.tensor.matmul(warm_ps[:], lhsT=Zb_sb[:, 0:128], rhs=Zb_sb[:],
                             start=True, stop=True)

        for it in range(NT):
            xt_sb = xt_tiles[it]
            z = []
            for a in range(4):
                za = ps_z[a].tile([128, 256], F32, tag=f"z{a}")
                for j in range(2):
                    g = 2 * a + j
                    nc.tensor.matmul(
                        za[:],
                        lhsT=xt_sb[:, g * 128:(g + 1) * 128],
                        rhs=W_sb[:, j * 256:(j + 1) * 256],
                        start=(j == 0),
                        stop=(j == 1),
                    )
                z.append(za)

            # H4 butterfly over the a axis. All four z's are staged to SBUF
            # as bf16 (3 copies on ACT, 1 on DVE) so both butterfly stages
            # run all-SBUF in 16-bit (DVE 4x mode, Pool-eligible); work is
            # spread over ACT/DVE/Pool, each under the per-tile DMA budget.
            zs = []
            for a in range(4):
                t = zs_pool.tile([128, 256], BF16, tag=f"zs{a}")
                if a == 3:
                    nc.vector.tensor_copy(t[:], z[a][:])
                else:
                    nc.scalar.copy(t[:], z[a][:])
                zs.append(t)

            w0 = wb_pool.tile([128, 256], BF16, tag="w0")
            w1 = wb_pool.tile([128, 256], BF16, tag="w1")
            w2 = wb_pool.tile([128, 256], BF16, tag="w2")
            w3 = wb_pool.tile([128, 256], BF16, tag="w3")
            nc.vector.tensor_add(w0[:], zs[0][:], zs[2][:])
            nc.vector.tensor_sub(w2[:], zs[0][:], zs[2][:])
            nc.vector.tensor_add(w1[:], zs[1][:], zs[3][:])
            nc.vector.tensor_sub(w3[:], zs[1][:], zs[3][:])

            ob = out_pool.tile([128, D], BF16, tag="ob")
            row = o_d[it * 128:(it + 1) * 128, :]
            # dist-1 stage: DVE takes y0/y1 (4x bf16 mode), Pool takes y2/y3.
            # Quarter-stores ship on SP as soon as each y_c lands.
            nc.vector.tensor_add(ob[:, 0:256], w0[:], w1[:])
            nc.sync.dma_start(row[:, 0:256], ob[:, 0:256])
            nc.vector.tensor_sub(ob[:, 256:512], w0[:], w1[:])
            nc.sync.dma_start(row[:, 256:512], ob[:, 256:512])
            nc.gpsimd.tensor_add(ob[:, 512:768], w2[:], w3[:])
            nc.sync.dma_start(row[:, 512:768], ob[:, 512:768])
            nc.gpsimd.tensor_sub(ob[:, 768:1024], w2[:], w3[:])
            nc.sync.dma_start(row[:, 768:1024], ob[:, 768:1024])

    nc.compile()
    return nc


def _get_nc():
    if "nc" not in _cache:
        _cache["nc"] = _build_nc()
    return _cache["nc"]


def kernel(x, H, **_ignored):
    import ml_dtypes

    x = np.asarray(x, dtype=np.float32)
    H = np.asarray(H, dtype=np.float32)
    nc = _get_nc()

    # Derive the Kronecker factors from the given H (exact when H has the
    # Hadamard structure); fold in the 1/sqrt(1024) scale.
    R = np.ascontiguousarray(H[:128, :128]) * np.float32(1.0 / 32.0)  # symmetric
    H2s = np.ascontiguousarray(H[:2, :2])  # (-1)^popcount(i&j) signs
    # W2[b', j*256 + e8*128 + e_lo] = H2s[e8, j] * R[b', e_lo]
    W = np.ascontiguousarray(
        np.einsum("ej,bl->bjel", H2s, R).reshape(128, 512)
    ).astype(ml_dtypes.bfloat16)

    # Round x to bf16 (the on-chip pipeline would do the same before the
    # 16-bit matmuls) and pre-transpose per 128-row tile:
    # xt[t, b', g, n] = x[t, n, g, b']
    xb = x.reshape(ROWS // 128, 128, 8, 128).astype(ml_dtypes.bfloat16)
    xt = np.ascontiguousarray(xb.transpose(0, 3, 2, 1)).reshape(ROWS, D)

    in_maps = []
    for c in range(N_CORES):
        in_maps.append({
            "xt": np.ascontiguousarray(xt[c * SHARD:(c + 1) * SHARD]),
            "w": W,
        })

    res = bass_utils.run_bass_kernel_spmd(nc, in_maps, core_ids=list(range(N_CORES)))
    y = np.empty((ROWS, D, 2), dtype=np.float32)
    for c in range(N_CORES):
        y[c * SHARD:(c + 1) * SHARD, :, 0] = res.results[c]["out"].astype(np.float32)
    y[:, :, 1] = 0.0
    return y.reshape(B, S, D, 2)


# revision 17
# speedup vs baseline: 1.2511x; 1.2511x over previous
"""Hadamard transform kernel for Trainium2 (8 NeuronCores, SPMD data-parallel).

Computes y = (x @ H^T) / sqrt(D), padded with a zero imaginary plane ->
[B, S, D, 2], for x [4, 4096, 1024] fp32 and H the 1024-point Hadamard
matrix (H[i,j] = (-1)^popcount(i&j), symmetric, Kronecker-structured).

Precision/layout choices (all inside kernel(), tolerance is 2e-2):
  - x is rounded to bf16 and pre-transposed per 128-row tile on the host
    during sharding (pure layout + the same rounding the on-chip pipeline
    would apply): halves load traffic and removes all PE transposes.
  - The device writes the real plane in bf16 (host upcasts to fp32 and
    interleaves the zero imaginary plane): halves store traffic.
  Measured end-to-end relative error ~3e-3.

Per-core traffic: 4 MiB in + 4 MiB out + 0.13 MiB weights (~24 us at the
360 GB/s DMA roofline); every engine stage fits under the per-tile DMA
budget, so the kernel is DMA-bound.

Math (shard of 2048 rows, 16 row-tiles of 128):
  H_1024 = H_4 (x) H_256  under d = a*256 + b, f = c*256 + e, with
  H_256[e, j*128+b'] = H2[e8, j] * H128[e_lo, b'] (e = e8*128 + e_lo).
  Stage 1 (PE, bf16): per quarter a in 0..4, 2 accumulating matmuls
    z_a += xt[:, (2a+j)*128:...]^T @ W2[:, j*256:(j+1)*256], where
    W2[b', j*256 + e8*128 + e_lo] = H2[e8,j] * H128[e_lo,b'] / 32
    (host-precomputed, exact +-2^-5 entries, bf16).
  Stage 2 (H4 butterfly over a, 256 cols/op, bf16 intermediates):
    stage z0,z1 -> SBUF (ACT), then dist-2: w0=z0+z2 (ACT), w1=z1+z3,
    w2=z0-z2, w3=z1-z3 (DVE); dist-1: y0=w0+w1, y1=w0-w1 (DVE, 4x bf16),
    y2=w2+w3, y3=w2-w3 (Pool). Quarter-stores ship as each y_c lands.
  Startup: all 16 loads queued on SP up front; W rides the ACT queue; a
  burst of dummy matmuls ramps the PE p-state during the first loads.
"""

import numpy as np
from contextlib import ExitStack

import concourse.bass as bass
import concourse.tile as tile
from concourse import bacc, bass_utils, mybir

N_CORES = 8
B, S, D = 4, 4096, 1024
ROWS = B * S                 # 16384
SHARD = ROWS // N_CORES      # 2048
NT = SHARD // 128            # 16 tiles of 128 rows per core
F32 = mybir.dt.float32
BF16 = mybir.dt.bfloat16

_cache = {}

CFG = {
    "xin_bufs": 16,
    "out_bufs": 4,
    "zs_bufs": 3,
    "w_bufs": 3,
    "z_bufs": 2,
    "warmup": 10,
}


def _build_nc(cfg=None):
    cfg = {**CFG, **(cfg or {})}
    nc = bacc.Bacc("TRN2", target_bir_lowering=False, debug=False)
    # xt: per tile t, xt[t*128+b', g*128+n] = x[t*128+n, g*128+b'] (bf16)
    xt_d = nc.dram_tensor("xt", [SHARD, D], BF16, kind="ExternalInput").ap()
    w_d = nc.dram_tensor("w", [128, 512], BF16, kind="ExternalInput").ap()
    o_d = nc.dram_tensor("out", [SHARD, D], BF16, kind="ExternalOutput").ap()

    with tile.TileContext(nc) as tc, ExitStack() as ctx:
        const_pool = ctx.enter_context(tc.tile_pool(name="const", bufs=1))
        xin_pool = ctx.enter_context(tc.tile_pool(name="xin", bufs=cfg["xin_bufs"]))
        out_pool = ctx.enter_context(tc.tile_pool(name="outp", bufs=cfg["out_bufs"]))
        zs_pool = ctx.enter_context(tc.tile_pool(name="zs", bufs=cfg["zs_bufs"]))
        wb_pool = ctx.enter_context(tc.tile_pool(name="wb", bufs=cfg["w_bufs"]))
        ps_z = [
            ctx.enter_context(
                tc.tile_pool(name=f"ps_z{a}", bufs=cfg["z_bufs"], space="PSUM"))
            for a in range(4)
        ]

        # All 16 xt loads queued on SP up front; W rides the ACT queue.
        xt_tiles = []
        for it in range(NT):
            xt_sb = xin_pool.tile([128, D], BF16, tag="xt")
            nc.sync.dma_start(xt_sb[:], xt_d[it * 128:(it + 1) * 128, :])
            xt_tiles.append(xt_sb)

        W_sb = const_pool.tile([128, 512], BF16, tag="W")
        nc.scalar.dma_start(W_sb[:], w_d[:])

        # PE p-state warmup: dummy matmuls on a zeroed tile while the first
        # loads are in flight (reusing the z0 PSUM pool).
        Zb_sb = const_pool.tile([128, 256], BF16, tag="Zb")
        nc.vector.memset(Zb_sb[:], 0.0)
        for _ in range(cfg["warmup"]):
            warm_ps = ps_z[0].tile([128, 256], F32, tag="z0")
            nc.tensor.matmul(warm_ps[:], lhsT=Zb_sb[:, 0:128], rhs=Zb_sb[:],
                             start=True, stop=True)

        for it in range(NT):
            xt_sb = xt_tiles[it]
            z = []
            for a in range(4):
                za = ps_z[a].tile([128, 256], F32, tag=f"z{a}")
                for j in range(2):
                    g = 2 * a + j
                    nc.tensor.matmul(
                        za[:],
                        lhsT=xt_sb[:, g * 128:(g + 1) * 128],
                        rhs=W_sb[:, j * 256:(j + 1) * 256],
                        start=(j == 0),
                        stop=(j == 1),
                    )
                z.append(za)

            # H4 butterfly over the a axis. All four z's are staged to SBUF
            # as bf16 (3 copies on ACT, 1 on DVE) so both butterfly stages
            # run all-SBUF in 16-bit (DVE 4x mode, Pool-eligible); work is
            # spread over ACT/DVE/Pool, each under the per-tile DMA budget.
            zs = []
            for a in range(4):
                t = zs_pool.tile([128, 256], BF16, tag=f"zs{a}")
                if a == 3:
                    nc.vector.tensor_copy(t[:], z[a][:])
                else:
                    nc.scalar.copy(t[:], z[a][:])
                zs.append(t)

            w0 = wb_pool.tile([128, 256], BF16, tag="w0")
            w1 = wb_pool.tile([128, 256], BF16, tag="w1")
            w2 = wb_pool.tile([128, 256], BF16, tag="w2")
            w3 = wb_pool.tile([128, 256], BF16, tag="w3")
            nc.vector.tensor_add(w0[:], zs[0][:], zs[2][:])
            nc.vector.tensor_sub(w2[:], zs[0][:], zs[2][:])
            nc.vector.tensor_add(w1[:], zs[1][:], zs[3][:])
            nc.vector.tensor_sub(w3[:], zs[1][:], zs[3][:])

            ob = out_pool.tile([128, D], BF16, tag="ob")
            row = o_d[it * 128:(it + 1) * 128, :]
            # dist-1 stage: DVE takes y0/y1 (4x bf16 mode), Pool takes y2/y3.
            # The lo half-store rides SP (HWDGE); the hi half-store is issued
            # by the Pool engine itself (SWDGE) right after its own y3 - the
            # shared HWDGE generator (~625 ns/DMA) stays under budget.
            nc.vector.tensor_add(ob[:, 0:256], w0[:], w1[:])
            nc.vector.tensor_sub(ob[:, 256:512], w0[:], w1[:])
            nc.sync.dma_start(row[:, 0:512], ob[:, 0:512])
            nc.gpsimd.tensor_add(ob[:, 512:768], w2[:], w3[:])
            nc.gpsimd.tensor_sub(ob[:, 768:1024], w2[:], w3[:])
            nc.gpsimd.dma_start(row[:, 512:1024], ob[:, 512:1024])

    nc.compile()
    return nc


def _get_nc():
    if "nc" not in _cache:
        _cache["nc"] = _build_nc()
    return _cache["nc"]


def kernel(x, H, **_ignored):
    import ml_dtypes

    x = np.asarray(x, dtype=np.float32)
    H = np.asarray(H, dtype=np.float32)
    nc = _get_nc()

    # Derive the Kronecker factors from the given H (exact when H has the
    # Hadamard structure); fold in the 1/sqrt(1024) scale.
    R = np.ascontiguousarray(H[:128, :128]) * np.float32(1.0 / 32.0)  # symmetric
    H2s = np.ascontiguousarray(H[:2, :2])  # (-1)^popcount(i&j) signs
    # W2[b', j*256 + e8*128 + e_lo] = H2s[e8, j] * R[b', e_lo]
    W = np.ascontiguousarray(
        np.einsum("ej,bl->bjel", H2s, R).reshape(128, 512)
    ).astype(ml_dtypes.bfloat16)

    # Round x to bf16 (the on-chip pipeline would do the same before the
    # 16-bit matmuls) and pre-transpose per 128-row tile:
    # xt[t, b', g, n] = x[t, n, g, b']
    xb = x.reshape(ROWS // 128, 128, 8, 128).astype(ml_dtypes.bfloat16)
    xt = np.ascontiguousarray(xb.transpose(0, 3, 2, 1)).reshape(ROWS, D)

    in_maps = []
    for c in range(N_CORES):
        in_maps.append({
            "xt": np.ascontiguousarray(xt[c * SHARD:(c + 1) * SHARD]),
            "w": W,
        })

    res = bass_utils.run_bass_kernel_spmd(nc, in_maps, core_ids=list(range(N_CORES)))
    y = np.empty((ROWS, D, 2), dtype=np.float32)
    for c in range(N_CORES):
        y[c * SHARD:(c + 1) * SHARD, :, 0] = res.results[c]["out"].astype(np.float32)
    y[:, :, 1] = 0.0
    return y.reshape(B, S, D, 2)


# revision 18
# speedup vs baseline: 1.6516x; 1.3202x over previous
"""Hadamard transform kernel for Trainium2 (8 NeuronCores, SPMD data-parallel).

Computes y = (x @ H^T) / sqrt(D), padded with a zero imaginary plane ->
[B, S, D, 2], for x [4, 4096, 1024] fp32 and H the 1024-point Hadamard
matrix (H[i,j] = (-1)^popcount(i&j), symmetric, Kronecker-structured).

Precision/layout choices (all inside kernel(), tolerance is 2e-2):
  - x is rounded to bf16 and pre-transposed per 128-row tile on the host
    during sharding (pure layout + the same rounding the on-chip pipeline
    would apply): halves load traffic and removes all PE transposes.
  - The device writes the real plane in bf16 (host upcasts to fp32 and
    interleaves the zero imaginary plane): halves store traffic.
  Measured end-to-end relative error ~3e-3.

Per-core traffic: 4 MiB in + 4 MiB out + 0.13 MiB weights (~24 us at the
360 GB/s DMA roofline); every engine stage fits under the per-tile DMA
budget, so the kernel is DMA-bound.

Math (shard of 2048 rows, 16 row-tiles of 128):
  H_1024 = H_4 (x) H_256  under d = a*256 + b, f = c*256 + e, with
  H_256[e, j*128+b'] = H2[e8, j] * H128[e_lo, b'] (e = e8*128 + e_lo).
  Stage 1 (PE, bf16): per quarter a in 0..4, 2 accumulating matmuls
    z_a += xt[:, (2a+j)*128:...]^T @ W2[:, j*256:(j+1)*256], where
    W2[b', j*256 + e8*128 + e_lo] = H2[e8,j] * H128[e_lo,b'] / 32
    (host-precomputed, exact +-2^-5 entries, bf16).
  Stage 2 (H4 butterfly over a, 256 cols/op, bf16 intermediates):
    stage z0,z1 -> SBUF (ACT), then dist-2: w0=z0+z2 (ACT), w1=z1+z3,
    w2=z0-z2, w3=z1-z3 (DVE); dist-1: y0=w0+w1, y1=w0-w1 (DVE, 4x bf16),
    y2=w2+w3, y3=w2-w3 (Pool). Quarter-stores ship as each y_c lands.
  Startup: all 16 loads queued on SP up front; W rides the ACT queue; a
  burst of dummy matmuls ramps the PE p-state during the first loads.
"""

import numpy as np
from contextlib import ExitStack

import concourse.bass as bass
import concourse.tile as tile
from concourse import bacc, bass_utils, mybir

N_CORES = 8
B, S, D = 4, 4096, 1024
ROWS = B * S                 # 16384
SHARD = ROWS // N_CORES      # 2048
NT = SHARD // 128            # 16 tiles of 128 rows per core
F32 = mybir.dt.float32
BF16 = mybir.dt.bfloat16

_cache = {}

CFG = {
    "xin_bufs": 16,
    "out_bufs": 4,
    "zs_bufs": 3,
    "w_bufs": 3,
    "z_bufs": 2,
    "warmup": 10,
}


def _build_nc(cfg=None):
    cfg = {**CFG, **(cfg or {})}
    nc = bacc.Bacc("TRN2", target_bir_lowering=False, debug=False)
    # xt: per tile t, xt[t*128+b', g*128+n] = x[t*128+n, g*128+b'] (bf16)
    xt_d = nc.dram_tensor("xt", [SHARD, D], BF16, kind="ExternalInput").ap()
    w_d = nc.dram_tensor("w", [128, 512], BF16, kind="ExternalInput").ap()
    o_d = nc.dram_tensor("out", [SHARD, D], BF16, kind="ExternalOutput").ap()

    with tile.TileContext(nc) as tc, ExitStack() as ctx:
        const_pool = ctx.enter_context(tc.tile_pool(name="const", bufs=1))
        xin_pool = ctx.enter_context(tc.tile_pool(name="xin", bufs=cfg["xin_bufs"]))
        out_pool = ctx.enter_context(tc.tile_pool(name="outp", bufs=cfg["out_bufs"]))
        zs_pool = ctx.enter_context(tc.tile_pool(name="zs", bufs=cfg["zs_bufs"]))
        wb_pool = ctx.enter_context(tc.tile_pool(name="wb", bufs=cfg["w_bufs"]))
        ps_z = [
            ctx.enter_context(
                tc.tile_pool(name=f"ps_z{a}", bufs=cfg["z_bufs"], space="PSUM"))
            for a in range(4)
        ]

        # All 16 xt loads queued on SP up front; W rides the ACT queue.
        xt_tiles = []
        for it in range(NT):
            xt_sb = xin_pool.tile([128, D], BF16, tag="xt")
            nc.sync.dma_start(xt_sb[:], xt_d[it * 128:(it + 1) * 128, :])
            xt_tiles.append(xt_sb)

        W_sb = const_pool.tile([128, 512], BF16, tag="W")
        nc.scalar.dma_start(W_sb[:], w_d[:])

        # PE p-state warmup: dummy matmuls on a zeroed tile while the first
        # loads are in flight (reusing the z0 PSUM pool).
        Zb_sb = const_pool.tile([128, 256], BF16, tag="Zb")
        nc.vector.memset(Zb_sb[:], 0.0)
        for _ in range(cfg["warmup"]):
            warm_ps = ps_z[0].tile([128, 256], F32, tag="z0")
            nc.tensor.matmul(warm_ps[:], lhsT=Zb_sb[:, 0:128], rhs=Zb_sb[:],
                             start=True, stop=True)

        for it in range(NT):
            xt_sb = xt_tiles[it]
            z = []
            for a in range(4):
                za = ps_z[a].tile([128, 256], F32, tag=f"z{a}")
                for j in range(2):
                    g = 2 * a + j
                    nc.tensor.matmul(
                        za[:],
                        lhsT=xt_sb[:, g * 128:(g + 1) * 128],
                        rhs=W_sb[:, j * 256:(j + 1) * 256],
                        start=(j == 0),
                        stop=(j == 1),
                    )
                z.append(za)

            # H4 butterfly over the a axis. All four z's are staged to SBUF
            # as bf16 (3 copies on ACT, 1 on DVE) so both butterfly stages
            # run all-SBUF in 16-bit (DVE 4x mode, Pool-eligible); work is
            # spread over ACT/DVE/Pool, each under the per-tile DMA budget.
            zs = []
            for a in range(4):
                t = zs_pool.tile([128, 256], BF16, tag=f"zs{a}")
                if a == 3:
                    nc.vector.tensor_copy(t[:], z[a][:])
                else:
                    nc.scalar.copy(t[:], z[a][:])
                zs.append(t)

            w0 = wb_pool.tile([128, 256], BF16, tag="w0")
            w1 = wb_pool.tile([128, 256], BF16, tag="w1")
            w2 = wb_pool.tile([128, 256], BF16, tag="w2")
            w3 = wb_pool.tile([128, 256], BF16, tag="w3")
            nc.vector.tensor_add(w0[:], zs[0][:], zs[2][:])
            nc.vector.tensor_sub(w2[:], zs[0][:], zs[2][:])
            nc.vector.tensor_add(w1[:], zs[1][:], zs[3][:])
            nc.vector.tensor_sub(w3[:], zs[1][:], zs[3][:])

            ob = out_pool.tile([128, D], BF16, tag="ob")
            row = o_d[it * 128:(it + 1) * 128, :]
            # dist-1 stage: DVE takes y0/y1 (4x bf16 mode), Pool takes y2/y3.
            # One full store per tile on SP: the shared HWDGE generator costs
            # ~625 ns per DMA, so instruction count matters more than shipping
            # halves early (SWDGE would burn ~1 us of Pool ENGINE per store).
            nc.vector.tensor_add(ob[:, 0:256], w0[:], w1[:])
            nc.vector.tensor_sub(ob[:, 256:512], w0[:], w1[:])
            nc.gpsimd.tensor_add(ob[:, 512:768], w2[:], w3[:])
            nc.gpsimd.tensor_sub(ob[:, 768:1024], w2[:], w3[:])
            nc.sync.dma_start(row[:], ob[:])

    nc.compile()
    return nc


def _get_nc():
    if "nc" not in _cache:
        _cache["nc"] = _build_nc()
    return _cache["nc"]


def kernel(x, H, **_ignored):
    import ml_dtypes

    x = np.asarray(x, dtype=np.float32)
    H = np.asarray(H, dtype=np.float32)
    nc = _get_nc()

    # Derive the Kronecker factors from the given H (exact when H has the
    # Hadamard structure); fold in the 1/sqrt(1024) scale.
    R = np.ascontiguousarray(H[:128, :128]) * np.float32(1.0 / 32.0)  # symmetric
    H2s = np.ascontiguousarray(H[:2, :2])  # (-1)^popcount(i&j) signs
    # W2[b', j*256 + e8*128 + e_lo] = H2s[e8, j] * R[b', e_lo]
    W = np.ascontiguousarray(
        np.einsum("ej,bl->bjel", H2s, R).reshape(128, 512)
    ).astype(ml_dtypes.bfloat16)

    # Round x to bf16 (the on-chip pipeline would do the same before the
    # 16-bit matmuls) and pre-transpose per 128-row tile:
    # xt[t, b', g, n] = x[t, n, g, b']
    xb = x.reshape(ROWS // 128, 128, 8, 128).astype(ml_dtypes.bfloat16)
    xt = np.ascontiguousarray(xb.transpose(0, 3, 2, 1)).reshape(ROWS, D)

    in_maps = []
    for c in range(N_CORES):
        in_maps.append({
            "xt": np.ascontiguousarray(xt[c * SHARD:(c + 1) * SHARD]),
            "w": W,
        })

    res = bass_utils.run_bass_kernel_spmd(nc, in_maps, core_ids=list(range(N_CORES)))
    y = np.empty((ROWS, D, 2), dtype=np.float32)
    for c in range(N_CORES):
        y[c * SHARD:(c + 1) * SHARD, :, 0] = res.results[c]["out"].astype(np.float32)
    y[:, :, 1] = 0.0
    return y.reshape(B, S, D, 2)


# revision 19
# speedup vs baseline: 1.7405x; 1.0538x over previous
"""Hadamard transform kernel for Trainium2 (8 NeuronCores, SPMD data-parallel).

Computes y = (x @ H^T) / sqrt(D), padded with a zero imaginary plane ->
[B, S, D, 2], for x [4, 4096, 1024] fp32 and H the 1024-point Hadamard
matrix (H[i,j] = (-1)^popcount(i&j), symmetric, Kronecker-structured).

Precision/layout choices (all inside kernel(), tolerance is 2e-2):
  - x is rounded to bf16 and pre-transposed per 128-row tile on the host
    during sharding (pure layout + the same rounding the on-chip pipeline
    would apply): halves load traffic and removes all PE transposes.
  - The device writes the real plane in bf16 (host upcasts to fp32 and
    interleaves the zero imaginary plane): halves store traffic.
  Measured end-to-end relative error ~3e-3.

Per-core traffic: 4 MiB in + 4 MiB out + 0.13 MiB weights (~24 us at the
360 GB/s DMA roofline); every engine stage fits under the per-tile DMA
budget, so the kernel is DMA-bound.

Math (shard of 2048 rows, 16 row-tiles of 128):
  H_1024 = H_4 (x) H_256  under d = a*256 + b, f = c*256 + e, with
  H_256[e, j*128+b'] = H2[e8, j] * H128[e_lo, b'] (e = e8*128 + e_lo).
  Stage 1 (PE, bf16): per quarter a in 0..4, 2 accumulating matmuls
    z_a += xt[:, (2a+j)*128:...]^T @ W2[:, j*256:(j+1)*256], where
    W2[b', j*256 + e8*128 + e_lo] = H2[e8,j] * H128[e_lo,b'] / 32
    (host-precomputed, exact +-2^-5 entries, bf16).
  Stage 2 (H4 butterfly over a, 256 cols/op, bf16 intermediates):
    stage z0,z1 -> SBUF (ACT), then dist-2: w0=z0+z2 (ACT), w1=z1+z3,
    w2=z0-z2, w3=z1-z3 (DVE); dist-1: y0=w0+w1, y1=w0-w1 (DVE, 4x bf16),
    y2=w2+w3, y3=w2-w3 (Pool). Quarter-stores ship as each y_c lands.
  Startup: all 16 loads queued on SP up front; W rides the ACT queue; a
  burst of dummy matmuls ramps the PE p-state during the first loads.
"""

import numpy as np
from contextlib import ExitStack

import concourse.bass as bass
import concourse.tile as tile
from concourse import bacc, bass_utils, mybir

N_CORES = 8
B, S, D = 4, 4096, 1024
ROWS = B * S                 # 16384
SHARD = ROWS // N_CORES      # 2048
NT = SHARD // 128            # 16 tiles of 128 rows per core
F32 = mybir.dt.float32
BF16 = mybir.dt.bfloat16

_cache = {}

CFG = {
    "xin_bufs": 16,
    "out_bufs": 4,
    "zs_bufs": 3,
    "w_bufs": 3,
    "z_bufs": 2,
    "warmup": 10,
}


def _build_nc(cfg=None):
    cfg = {**CFG, **(cfg or {})}
    nc = bacc.Bacc("TRN2", target_bir_lowering=False, debug=False)
    # xt: per tile t, xt[t*128+b', g*128+n] = x[t*128+n, g*128+b'] (bf16)
    xt_d = nc.dram_tensor("xt", [SHARD, D], BF16, kind="ExternalInput").ap()
    w_d = nc.dram_tensor("w", [128, 512], BF16, kind="ExternalInput").ap()
    o_d = nc.dram_tensor("out", [SHARD, D], BF16, kind="ExternalOutput").ap()

    with tile.TileContext(nc) as tc, ExitStack() as ctx:
        const_pool = ctx.enter_context(tc.tile_pool(name="const", bufs=1))
        xin_pool = ctx.enter_context(tc.tile_pool(name="xin", bufs=cfg["xin_bufs"]))
        out_pool = ctx.enter_context(tc.tile_pool(name="outp", bufs=cfg["out_bufs"]))
        zs_pool = ctx.enter_context(tc.tile_pool(name="zs", bufs=cfg["zs_bufs"]))
        wb_pool = ctx.enter_context(tc.tile_pool(name="wb", bufs=cfg["w_bufs"]))
        ps_z01 = ctx.enter_context(
            tc.tile_pool(name="ps_z01", bufs=cfg["z_bufs"], space="PSUM"))
        ps_z23 = ctx.enter_context(
            tc.tile_pool(name="ps_z23", bufs=cfg["z_bufs"], space="PSUM"))

        # All 16 xt loads queued on SP up front; W rides the ACT queue.
        xt_tiles = []
        for it in range(NT):
            xt_sb = xin_pool.tile([128, D], BF16, tag="xt")
            nc.sync.dma_start(xt_sb[:], xt_d[it * 128:(it + 1) * 128, :])
            xt_tiles.append(xt_sb)

        W_sb = const_pool.tile([128, 512], BF16, tag="W")
        nc.scalar.dma_start(W_sb[:], w_d[:])

        # PE p-state warmup: dummy matmuls on a zeroed tile while the first
        # loads are in flight (reusing the z0 PSUM pool).
        Zb_sb = const_pool.tile([128, 256], BF16, tag="Zb")
        nc.vector.memset(Zb_sb[:], 0.0)
        for _ in range(cfg["warmup"]):
            warm_ps = ps_z01.tile([128, 512], F32, tag="z01")
            nc.tensor.matmul(warm_ps[:, 0:256], lhsT=Zb_sb[:, 0:128], rhs=Zb_sb[:],
                             start=True, stop=True)

        for it in range(NT):
            xt_sb = xt_tiles[it]
            last = it == NT - 1
            # z0/z1 share one PSUM bank (two accumulation groups), likewise
            # z2/z3: staging to SBUF is then just TWO 512-wide ACT copies.
            z01 = ps_z01.tile([128, 512], F32, tag="z01")
            z23 = ps_z23.tile([128, 512], F32, tag="z23")
            zs01 = zs_pool.tile([128, 512], BF16, tag="zs01")
            zs23 = zs_pool.tile([128, 512], BF16, tag="zs23")
            for a in range(4):
                zps = (z01 if a < 2 else z23)[:, (a % 2) * 256:(a % 2 + 1) * 256]
                for j in range(2):
                    g = 2 * a + j
                    nc.tensor.matmul(
                        zps,
                        lhsT=xt_sb[:, g * 128:(g + 1) * 128],
                        rhs=W_sb[:, j * 256:(j + 1) * 256],
                        start=(j == 0),
                        stop=(j == 1),
                    )
                if a == 1:
                    nc.scalar.copy(zs01[:], z01[:])
                elif a == 3:
                    nc.scalar.copy(zs23[:], z23[:])

            # H4 butterfly over the a axis, all-SBUF in bf16 (DVE 4x mode,
            # Pool-eligible). zs01 = [z0|z1], zs23 = [z2|z3].
            w0 = wb_pool.tile([128, 256], BF16, tag="w0")
            w1 = wb_pool.tile([128, 256], BF16, tag="w1")
            w2 = wb_pool.tile([128, 256], BF16, tag="w2")
            w3 = wb_pool.tile([128, 256], BF16, tag="w3")
            nc.vector.tensor_add(w0[:], zs01[:, 0:256], zs23[:, 0:256])
            nc.vector.tensor_sub(w2[:], zs01[:, 0:256], zs23[:, 0:256])
            nc.vector.tensor_add(w1[:], zs01[:, 256:512], zs23[:, 256:512])
            nc.vector.tensor_sub(w3[:], zs01[:, 256:512], zs23[:, 256:512])

            ob = out_pool.tile([128, D], BF16, tag="ob")
            row = o_d[it * 128:(it + 1) * 128, :]
            # dist-1 stage: DVE takes y0/y1 (4x bf16 mode), Pool takes y2/y3.
            # One full store per tile on SP: the shared HWDGE generator costs
            # ~625 ns per DMA, so instruction count matters more than shipping
            # halves early (SWDGE would burn ~1 us of Pool ENGINE per store).
            # The final tile keeps everything on DVE and splits its store so
            # the drain tail is as short as possible.
            nc.vector.tensor_add(ob[:, 0:256], w0[:], w1[:])
            nc.vector.tensor_sub(ob[:, 256:512], w0[:], w1[:])
            if last:
                nc.sync.dma_start(row[:, 0:512], ob[:, 0:512])
                nc.vector.tensor_add(ob[:, 512:768], w2[:], w3[:])
                nc.vector.tensor_sub(ob[:, 768:1024], w2[:], w3[:])
                nc.sync.dma_start(row[:, 512:1024], ob[:, 512:1024])
            else:
                nc.gpsimd.tensor_add(ob[:, 512:768], w2[:], w3[:])
                nc.gpsimd.tensor_sub(ob[:, 768:1024], w2[:], w3[:])
                nc.sync.dma_start(row[:], ob[:])

    nc.compile()
    return nc


def _get_nc():
    if "nc" not in _cache:
        _cache["nc"] = _build_nc()
    return _cache["nc"]


def kernel(x, H, **_ignored):
    import ml_dtypes

    x = np.asarray(x, dtype=np.float32)
    H = np.asarray(H, dtype=np.float32)
    nc = _get_nc()

    # Derive the Kronecker factors from the given H (exact when H has the
    # Hadamard structure); fold in the 1/sqrt(1024) scale.
    R = np.ascontiguousarray(H[:128, :128]) * np.float32(1.0 / 32.0)  # symmetric
    H2s = np.ascontiguousarray(H[:2, :2])  # (-1)^popcount(i&j) signs
    # W2[b', j*256 + e8*128 + e_lo] = H2s[e8, j] * R[b', e_lo]
    W = np.ascontiguousarray(
        np.einsum("ej,bl->bjel", H2s, R).reshape(128, 512)
    ).astype(ml_dtypes.bfloat16)

    # Round x to bf16 (the on-chip pipeline would do the same before the
    # 16-bit matmuls) and pre-transpose per 128-row tile:
    # xt[t, b', g, n] = x[t, n, g, b']
    xb = x.reshape(ROWS // 128, 128, 8, 128).astype(ml_dtypes.bfloat16)
    xt = np.ascontiguousarray(xb.transpose(0, 3, 2, 1)).reshape(ROWS, D)

    in_maps = []
    for c in range(N_CORES):
        in_maps.append({
            "xt": np.ascontiguousarray(xt[c * SHARD:(c + 1) * SHARD]),
            "w": W,
        })

    res = bass_utils.run_bass_kernel_spmd(nc, in_maps, core_ids=list(range(N_CORES)))
    y = np.empty((ROWS, D, 2), dtype=np.float32)
    for c in range(N_CORES):
        y[c * SHARD:(c + 1) * SHARD, :, 0] = res.results[c]["out"].astype(np.float32)
    y[:, :, 1] = 0.0
    return y.reshape(B, S, D, 2)


# revision 20
# speedup vs baseline: 1.8740x; 1.0767x over previous
"""Hadamard transform kernel for Trainium2 (8 NeuronCores, SPMD data-parallel).

Computes y = (x @ H^T) / sqrt(D), padded with a zero imaginary plane ->
[B, S, D, 2], for x [4, 4096, 1024] fp32 and H the 1024-point Hadamard
matrix (H[i,j] = (-1)^popcount(i&j), symmetric, Kronecker-structured).

Precision/layout choices (all inside kernel(), tolerance is 2e-2):
  - x is rounded to bf16 and pre-transposed per 128-row tile on the host
    during sharding (pure layout + the same rounding the on-chip pipeline
    would apply): halves load traffic and removes all PE transposes.
  - The device writes the real plane in bf16 (host upcasts to fp32 and
    interleaves the zero imaginary plane): halves store traffic.
  Measured end-to-end relative error ~3e-3.

Per-core traffic: 4 MiB in + 4 MiB out + 0.13 MiB weights (~24 us at the
360 GB/s DMA roofline); every engine stage fits under the per-tile DMA
budget, so the kernel is DMA-bound.

Math (shard of 2048 rows, 16 row-tiles of 128):
  H_1024 = H_4 (x) H_256  under d = a*256 + b, f = c*256 + e, with
  H_256[e, j*128+b'] = H2[e8, j] * H128[e_lo, b'] (e = e8*128 + e_lo).
  Stage 1 (PE, bf16): per quarter a in 0..4, 2 accumulating matmuls
    z_a += xt[:, (2a+j)*128:...]^T @ W2[:, j*256:(j+1)*256], where
    W2[b', j*256 + e8*128 + e_lo] = H2[e8,j] * H128[e_lo,b'] / 32
    (host-precomputed, exact +-2^-5 entries, bf16).
  Stage 2 (H4 butterfly over a, 256 cols/op, bf16 intermediates):
    stage z0,z1 -> SBUF (ACT), then dist-2: w0=z0+z2 (ACT), w1=z1+z3,
    w2=z0-z2, w3=z1-z3 (DVE); dist-1: y0=w0+w1, y1=w0-w1 (DVE, 4x bf16),
    y2=w2+w3, y3=w2-w3 (Pool). Quarter-stores ship as each y_c lands.
  Startup: all 16 loads queued on SP up front; W rides the ACT queue; a
  burst of dummy matmuls ramps the PE p-state during the first loads.
"""

import numpy as np
from contextlib import ExitStack

import concourse.bass as bass
import concourse.tile as tile
from concourse import bacc, bass_utils, mybir

N_CORES = 8
B, S, D = 4, 4096, 1024
ROWS = B * S                 # 16384
SHARD = ROWS // N_CORES      # 2048
NT = SHARD // 128            # 16 tiles of 128 rows per core
F32 = mybir.dt.float32
BF16 = mybir.dt.bfloat16

_cache = {}

CFG = {
    "xin_bufs": 16,
    "out_bufs": 6,
    "zs_bufs": 4,
    "w_bufs": 4,
    "z_bufs": 3,
    "warmup": 10,
}


def _build_nc(cfg=None):
    cfg = {**CFG, **(cfg or {})}
    nc = bacc.Bacc("TRN2", target_bir_lowering=False, debug=False)
    # xt: per tile t, xt[t*128+b', g*128+n] = x[t*128+n, g*128+b'] (bf16)
    xt_d = nc.dram_tensor("xt", [SHARD, D], BF16, kind="ExternalInput").ap()
    w_d = nc.dram_tensor("w", [128, 512], BF16, kind="ExternalInput").ap()
    o_d = nc.dram_tensor("out", [SHARD, D], BF16, kind="ExternalOutput").ap()

    with tile.TileContext(nc) as tc, ExitStack() as ctx:
        const_pool = ctx.enter_context(tc.tile_pool(name="const", bufs=1))
        xin_pool = ctx.enter_context(tc.tile_pool(name="xin", bufs=cfg["xin_bufs"]))
        out_pool = ctx.enter_context(tc.tile_pool(name="outp", bufs=cfg["out_bufs"]))
        zs_pool = ctx.enter_context(tc.tile_pool(name="zs", bufs=cfg["zs_bufs"]))
        wb_pool = ctx.enter_context(tc.tile_pool(name="wb", bufs=cfg["w_bufs"]))
        ps_z01 = ctx.enter_context(
            tc.tile_pool(name="ps_z01", bufs=cfg["z_bufs"], space="PSUM"))
        ps_z23 = ctx.enter_context(
            tc.tile_pool(name="ps_z23", bufs=cfg["z_bufs"], space="PSUM"))

        # All 16 xt loads queued on SP up front; W rides the ACT queue.
        xt_tiles = []
        for it in range(NT):
            xt_sb = xin_pool.tile([128, D], BF16, tag="xt")
            nc.sync.dma_start(xt_sb[:], xt_d[it * 128:(it + 1) * 128, :])
            xt_tiles.append(xt_sb)

        W_sb = const_pool.tile([128, 512], BF16, tag="W")
        nc.scalar.dma_start(W_sb[:], w_d[:])

        # PE p-state warmup: dummy matmuls on a zeroed tile while the first
        # loads are in flight (reusing the z0 PSUM pool).
        Zb_sb = const_pool.tile([128, 256], BF16, tag="Zb")
        nc.vector.memset(Zb_sb[:], 0.0)
        for _ in range(cfg["warmup"]):
            warm_ps = ps_z01.tile([128, 512], F32, tag="z01")
            nc.tensor.matmul(warm_ps[:, 0:256], lhsT=Zb_sb[:, 0:128], rhs=Zb_sb[:],
                             start=True, stop=True)

        for it in range(NT):
            xt_sb = xt_tiles[it]
            last = it == NT - 1
            # z0/z1 share one PSUM bank (two accumulation groups), likewise
            # z2/z3: staging to SBUF is then just TWO 512-wide ACT copies.
            z01 = ps_z01.tile([128, 512], F32, tag="z01")
            z23 = ps_z23.tile([128, 512], F32, tag="z23")
            zs01 = zs_pool.tile([128, 512], BF16, tag="zs01")
            zs23 = zs_pool.tile([128, 512], BF16, tag="zs23")
            for a in range(4):
                zps = (z01 if a < 2 else z23)[:, (a % 2) * 256:(a % 2 + 1) * 256]
                for j in range(2):
                    g = 2 * a + j
                    nc.tensor.matmul(
                        zps,
                        lhsT=xt_sb[:, g * 128:(g + 1) * 128],
                        rhs=W_sb[:, j * 256:(j + 1) * 256],
                        start=(j == 0),
                        stop=(j == 1),
                    )
                if a == 1:
                    nc.scalar.copy(zs01[:], z01[:])
                elif a == 3:
                    nc.scalar.copy(zs23[:], z23[:])

            # H4 butterfly over the a axis, all-SBUF in bf16 (DVE 4x mode,
            # Pool-eligible). zs01 = [z0|z1], zs23 = [z2|z3].
            w0 = wb_pool.tile([128, 256], BF16, tag="w0")
            w1 = wb_pool.tile([128, 256], BF16, tag="w1")
            w2 = wb_pool.tile([128, 256], BF16, tag="w2")
            w3 = wb_pool.tile([128, 256], BF16, tag="w3")
            nc.vector.tensor_add(w0[:], zs01[:, 0:256], zs23[:, 0:256])
            nc.vector.tensor_sub(w2[:], zs01[:, 0:256], zs23[:, 0:256])
            nc.vector.tensor_add(w1[:], zs01[:, 256:512], zs23[:, 256:512])
            nc.vector.tensor_sub(w3[:], zs01[:, 256:512], zs23[:, 256:512])

            ob = out_pool.tile([128, D], BF16, tag="ob")
            row = o_d[it * 128:(it + 1) * 128, :]
            # dist-1 stage: DVE takes y0/y1 (4x bf16 mode), Pool takes y2/y3.
            # One full store per tile on SP: the shared HWDGE generator costs
            # ~625 ns per DMA, so instruction count matters more than shipping
            # halves early (SWDGE would burn ~1 us of Pool ENGINE per store).
            # The final tile keeps everything on DVE and splits its store so
            # the drain tail is as short as possible.
            nc.vector.tensor_add(ob[:, 0:256], w0[:], w1[:])
            nc.vector.tensor_sub(ob[:, 256:512], w0[:], w1[:])
            if last:
                nc.sync.dma_start(row[:, 0:512], ob[:, 0:512])
                nc.vector.tensor_add(ob[:, 512:768], w2[:], w3[:])
                nc.vector.tensor_sub(ob[:, 768:1024], w2[:], w3[:])
                nc.sync.dma_start(row[:, 512:1024], ob[:, 512:1024])
            else:
                nc.gpsimd.tensor_add(ob[:, 512:768], w2[:], w3[:])
                nc.gpsimd.tensor_sub(ob[:, 768:1024], w2[:], w3[:])
                nc.sync.dma_start(row[:], ob[:])

    nc.compile()
    return nc


def _get_nc():
    if "nc" not in _cache:
        _cache["nc"] = _build_nc()
    return _cache["nc"]


def kernel(x, H, **_ignored):
    import ml_dtypes

    x = np.asarray(x, dtype=np.float32)
    H = np.asarray(H, dtype=np.float32)
    nc = _get_nc()

    # Derive the Kronecker factors from the given H (exact when H has the
    # Hadamard structure); fold in the 1/sqrt(1024) scale.
    R = np.ascontiguousarray(H[:128, :128]) * np.float32(1.0 / 32.0)  # symmetric
    H2s = np.ascontiguousarray(H[:2, :2])  # (-1)^popcount(i&j) signs
    # W2[b', j*256 + e8*128 + e_lo] = H2s[e8, j] * R[b', e_lo]
    W = np.ascontiguousarray(
        np.einsum("ej,bl->bjel", H2s, R).reshape(128, 512)
    ).astype(ml_dtypes.bfloat16)

    # Round x to bf16 (the on-chip pipeline would do the same before the
    # 16-bit matmuls) and pre-transpose per 128-row tile:
    # xt[t, b', g, n] = x[t, n, g, b']
    xb = x.reshape(ROWS // 128, 128, 8, 128).astype(ml_dtypes.bfloat16)
    xt = np.ascontiguousarray(xb.transpose(0, 3, 2, 1)).reshape(ROWS, D)

    in_maps = []
    for c in range(N_CORES):
        in_maps.append({
            "xt": np.ascontiguousarray(xt[c * SHARD:(c + 1) * SHARD]),
            "w": W,
        })

    res = bass_utils.run_bass_kernel_spmd(nc, in_maps, core_ids=list(range(N_CORES)))
    y = np.empty((ROWS, D, 2), dtype=np.float32)
    for c in range(N_CORES):
        y[c * SHARD:(c + 1) * SHARD, :, 0] = res.results[c]["out"].astype(np.float32)
    y[:, :, 1] = 0.0
    return y.reshape(B, S, D, 2)


# revision 21
# speedup vs baseline: 1.9619x; 1.0469x over previous
"""Hadamard transform kernel for Trainium2 (8 NeuronCores, SPMD data-parallel).

Computes y = (x @ H^T) / sqrt(D), padded with a zero imaginary plane ->
[B, S, D, 2], for x [4, 4096, 1024] fp32 and H the 1024-point Hadamard
matrix (H[i,j] = (-1)^popcount(i&j), symmetric, Kronecker-structured).

Precision/layout choices (all inside kernel(), tolerance is 2e-2):
  - x is rounded to bf16 and pre-transposed per 128-row tile on the host
    during sharding (pure layout + the same rounding the on-chip pipeline
    would apply): halves load traffic and removes all PE transposes.
  - The device writes the real plane in bf16 (host upcasts to fp32 and
    interleaves the zero imaginary plane): halves store traffic.
  Measured end-to-end relative error ~3e-3.

Per-core traffic: 4 MiB in + 4 MiB out + 0.13 MiB weights (~24 us at the
360 GB/s DMA roofline); every engine stage fits under the per-tile DMA
budget, so the kernel is DMA-bound.

Math (shard of 2048 rows, 16 row-tiles of 128):
  H_1024 = H_4 (x) H_256  under d = a*256 + b, f = c*256 + e, with
  H_256[e, j*128+b'] = H2[e8, j] * H128[e_lo, b'] (e = e8*128 + e_lo).
  Stage 1 (PE, bf16): per quarter a in 0..4, 2 accumulating matmuls
    z_a += xt[:, (2a+j)*128:...]^T @ W2[:, j*256:(j+1)*256], where
    W2[b', j*256 + e8*128 + e_lo] = H2[e8,j] * H128[e_lo,b'] / 32
    (host-precomputed, exact +-2^-5 entries, bf16).
  Stage 2 (H4 butterfly over a, 256 cols/op, bf16 intermediates):
    stage z0,z1 -> SBUF (ACT), then dist-2: w0=z0+z2 (ACT), w1=z1+z3,
    w2=z0-z2, w3=z1-z3 (DVE); dist-1: y0=w0+w1, y1=w0-w1 (DVE, 4x bf16),
    y2=w2+w3, y3=w2-w3 (Pool). Quarter-stores ship as each y_c lands.
  Startup: all 16 loads queued on SP up front; W rides the ACT queue; a
  burst of dummy matmuls ramps the PE p-state during the first loads.
"""

import numpy as np
from contextlib import ExitStack

import concourse.bass as bass
import concourse.tile as tile
from concourse import bacc, bass_utils, mybir

N_CORES = 8
B, S, D = 4, 4096, 1024
ROWS = B * S                 # 16384
SHARD = ROWS // N_CORES      # 2048
NT = SHARD // 128            # 16 tiles of 128 rows per core
F32 = mybir.dt.float32
BF16 = mybir.dt.bfloat16

_cache = {}

CFG = {
    "xin_bufs": 16,
    "out_bufs": 8,
    "zs_bufs": 6,
    "w_bufs": 6,
    "z_bufs": 4,
    "warmup": 10,
}


def _build_nc(cfg=None):
    cfg = {**CFG, **(cfg or {})}
    nc = bacc.Bacc("TRN2", target_bir_lowering=False, debug=False)
    # xt: per tile t, xt[t*128+b', g*128+n] = x[t*128+n, g*128+b'] (bf16)
    xt_d = nc.dram_tensor("xt", [SHARD, D], BF16, kind="ExternalInput").ap()
    w_d = nc.dram_tensor("w", [128, 512], BF16, kind="ExternalInput").ap()
    o_d = nc.dram_tensor("out", [SHARD, D], BF16, kind="ExternalOutput").ap()

    with tile.TileContext(nc) as tc, ExitStack() as ctx:
        const_pool = ctx.enter_context(tc.tile_pool(name="const", bufs=1))
        xin_pool = ctx.enter_context(tc.tile_pool(name="xin", bufs=cfg["xin_bufs"]))
        out_pool = ctx.enter_context(tc.tile_pool(name="outp", bufs=cfg["out_bufs"]))
        zs_pool = ctx.enter_context(tc.tile_pool(name="zs", bufs=cfg["zs_bufs"]))
        wb_pool = ctx.enter_context(tc.tile_pool(name="wb", bufs=cfg["w_bufs"]))
        ps_z01 = ctx.enter_context(
            tc.tile_pool(name="ps_z01", bufs=cfg["z_bufs"], space="PSUM"))
        ps_z23 = ctx.enter_context(
            tc.tile_pool(name="ps_z23", bufs=cfg["z_bufs"], space="PSUM"))

        # All 16 xt loads queued on SP up front; W rides the ACT queue.
        xt_tiles = []
        for it in range(NT):
            xt_sb = xin_pool.tile([128, D], BF16, tag="xt")
            nc.sync.dma_start(xt_sb[:], xt_d[it * 128:(it + 1) * 128, :])
            xt_tiles.append(xt_sb)

        W_sb = const_pool.tile([128, 512], BF16, tag="W")
        nc.scalar.dma_start(W_sb[:], w_d[:])

        # PE p-state warmup: dummy matmuls on a zeroed tile while the first
        # loads are in flight (reusing the z0 PSUM pool).
        Zb_sb = const_pool.tile([128, 256], BF16, tag="Zb")
        nc.vector.memset(Zb_sb[:], 0.0)
        for _ in range(cfg["warmup"]):
            warm_ps = ps_z01.tile([128, 512], F32, tag="z01")
            nc.tensor.matmul(warm_ps[:, 0:256], lhsT=Zb_sb[:, 0:128], rhs=Zb_sb[:],
                             start=True, stop=True)

        for it in range(NT):
            xt_sb = xt_tiles[it]
            last = it == NT - 1
            # z0/z1 share one PSUM bank (two accumulation groups), likewise
            # z2/z3: staging to SBUF is then just TWO 512-wide ACT copies.
            z01 = ps_z01.tile([128, 512], F32, tag="z01")
            z23 = ps_z23.tile([128, 512], F32, tag="z23")
            zs01 = zs_pool.tile([128, 512], BF16, tag="zs01")
            zs23 = zs_pool.tile([128, 512], BF16, tag="zs23")
            for a in range(4):
                zps = (z01 if a < 2 else z23)[:, (a % 2) * 256:(a % 2 + 1) * 256]
                for j in range(2):
                    g = 2 * a + j
                    nc.tensor.matmul(
                        zps,
                        lhsT=xt_sb[:, g * 128:(g + 1) * 128],
                        rhs=W_sb[:, j * 256:(j + 1) * 256],
                        start=(j == 0),
                        stop=(j == 1),
                    )
                if a == 1:
                    nc.scalar.copy(zs01[:], z01[:])
                elif a == 3:
                    nc.scalar.copy(zs23[:], z23[:])

            # H4 butterfly over the a axis, all-SBUF in bf16 (DVE 4x mode,
            # Pool-eligible). zs01 = [z0|z1], zs23 = [z2|z3].
            w0 = wb_pool.tile([128, 256], BF16, tag="w0")
            w1 = wb_pool.tile([128, 256], BF16, tag="w1")
            w2 = wb_pool.tile([128, 256], BF16, tag="w2")
            w3 = wb_pool.tile([128, 256], BF16, tag="w3")
            nc.vector.tensor_add(w0[:], zs01[:, 0:256], zs23[:, 0:256])
            nc.vector.tensor_sub(w2[:], zs01[:, 0:256], zs23[:, 0:256])
            nc.vector.tensor_add(w1[:], zs01[:, 256:512], zs23[:, 256:512])
            nc.vector.tensor_sub(w3[:], zs01[:, 256:512], zs23[:, 256:512])

            ob = out_pool.tile([128, D], BF16, tag="ob")
            row = o_d[it * 128:(it + 1) * 128, :]
            # dist-1 stage: DVE takes y0/y1 (4x bf16 mode), Pool takes y2/y3.
            # One full store per tile on SP: the shared HWDGE generator costs
            # ~625 ns per DMA, so instruction count matters more than shipping
            # halves early (SWDGE would burn ~1 us of Pool ENGINE per store).
            # The final tile keeps everything on DVE and splits its store so
            # the drain tail is as short as possible.
            nc.vector.tensor_add(ob[:, 0:256], w0[:], w1[:])
            nc.vector.tensor_sub(ob[:, 256:512], w0[:], w1[:])
            if last:
                nc.sync.dma_start(row[:, 0:512], ob[:, 0:512])
                nc.vector.tensor_add(ob[:, 512:768], w2[:], w3[:])
                nc.vector.tensor_sub(ob[:, 768:1024], w2[:], w3[:])
                nc.sync.dma_start(row[:, 512:1024], ob[:, 512:1024])
            else:
                nc.gpsimd.tensor_add(ob[:, 512:768], w2[:], w3[:])
                nc.gpsimd.tensor_sub(ob[:, 768:1024], w2[:], w3[:])
                nc.sync.dma_start(row[:], ob[:])

    nc.compile()
    return nc


def _get_nc():
    if "nc" not in _cache:
        _cache["nc"] = _build_nc()
    return _cache["nc"]


def kernel(x, H, **_ignored):
    import ml_dtypes

    x = np.asarray(x, dtype=np.float32)
    H = np.asarray(H, dtype=np.float32)
    nc = _get_nc()

    # Derive the Kronecker factors from the given H (exact when H has the
    # Hadamard structure); fold in the 1/sqrt(1024) scale.
    R = np.ascontiguousarray(H[:128, :128]) * np.float32(1.0 / 32.0)  # symmetric
    H2s = np.ascontiguousarray(H[:2, :2])  # (-1)^popcount(i&j) signs
    # W2[b', j*256 + e8*128 + e_lo] = H2s[e8, j] * R[b', e_lo]
    W = np.ascontiguousarray(
        np.einsum("ej,bl->bjel", H2s, R).reshape(128, 512)
    ).astype(ml_dtypes.bfloat16)

    # Round x to bf16 (the on-chip pipeline would do the same before the
    # 16-bit matmuls) and pre-transpose per 128-row tile:
    # xt[t, b', g, n] = x[t, n, g, b']
    xb = x.reshape(ROWS // 128, 128, 8, 128).astype(ml_dtypes.bfloat16)
    xt = np.ascontiguousarray(xb.transpose(0, 3, 2, 1)).reshape(ROWS, D)

    in_maps = []
    for c in range(N_CORES):
        in_maps.append({
            "xt": np.ascontiguousarray(xt[c * SHARD:(c + 1) * SHARD]),
            "w": W,
        })

    res = bass_utils.run_bass_kernel_spmd(nc, in_maps, core_ids=list(range(N_CORES)))
    y = np.empty((ROWS, D, 2), dtype=np.float32)
    for c in range(N_CORES):
        y[c * SHARD:(c + 1) * SHARD, :, 0] = res.results[c]["out"].astype(np.float32)
    y[:, :, 1] = 0.0
    return y.reshape(B, S, D, 2)


# revision 23
# speedup vs baseline: 1.9664x; 1.0023x over previous
"""Hadamard transform kernel for Trainium2 (8 NeuronCores, SPMD data-parallel).

Computes y = (x @ H^T) / sqrt(D), padded with a zero imaginary plane ->
[B, S, D, 2], for x [4, 4096, 1024] fp32 and H the 1024-point Hadamard
matrix (H[i,j] = (-1)^popcount(i&j), symmetric, Kronecker-structured).

Precision/layout choices (all inside kernel(), tolerance is 2e-2):
  - x is rounded to bf16 and pre-transposed per 128-row tile on the host
    during sharding (pure layout + the same rounding the on-chip pipeline
    would apply): halves load traffic and removes all PE transposes.
  - The device writes the real plane in bf16 (host upcasts to fp32 and
    interleaves the zero imaginary plane): halves store traffic.
  Measured end-to-end relative error ~3e-3.

Per-core traffic: 4 MiB in + 4 MiB out + 0.13 MiB weights (~24 us at the
360 GB/s DMA roofline); every engine stage fits under the per-tile DMA
budget, so the kernel is DMA-bound.

Math (shard of 2048 rows, 16 row-tiles of 128):
  H_1024 = H_4 (x) H_256  under d = a*256 + b, f = c*256 + e, with
  H_256[e, j*128+b'] = H2[e8, j] * H128[e_lo, b'] (e = e8*128 + e_lo).
  Stage 1 (PE, bf16): per quarter a in 0..4, 2 accumulating matmuls
    z_a += xt[:, (2a+j)*128:...]^T @ W2[:, j*256:(j+1)*256], where
    W2[b', j*256 + e8*128 + e_lo] = H2[e8,j] * H128[e_lo,b'] / 32
    (host-precomputed, exact +-2^-5 entries, bf16).
  Stage 2 (H4 butterfly over a, 256 cols/op, bf16 intermediates):
    stage z0,z1 -> SBUF (ACT), then dist-2: w0=z0+z2 (ACT), w1=z1+z3,
    w2=z0-z2, w3=z1-z3 (DVE); dist-1: y0=w0+w1, y1=w0-w1 (DVE, 4x bf16),
    y2=w2+w3, y3=w2-w3 (Pool). Quarter-stores ship as each y_c lands.
  Startup: all 16 loads queued on SP up front; W rides the ACT queue; a
  burst of dummy matmuls ramps the PE p-state during the first loads.
"""

import numpy as np
from contextlib import ExitStack

import concourse.bass as bass
import concourse.tile as tile
from concourse import bacc, bass_utils, mybir

N_CORES = 8
B, S, D = 4, 4096, 1024
ROWS = B * S                 # 16384
SHARD = ROWS // N_CORES      # 2048
NT = SHARD // 128            # 16 tiles of 128 rows per core
F32 = mybir.dt.float32
BF16 = mybir.dt.bfloat16

_cache = {}

CFG = {
    "xin_bufs": 16,
    "out_bufs": 8,
    "zs_bufs": 6,
    "w_bufs": 6,
    "z_bufs": 2,
    "warmup": 10,
}


def _build_nc(cfg=None):
    cfg = {**CFG, **(cfg or {})}
    nc = bacc.Bacc("TRN2", target_bir_lowering=False, debug=False)
    # xt: per tile t, xt[t*128+b', g*128+n] = x[t*128+n, g*128+b'] (bf16)
    xt_d = nc.dram_tensor("xt", [SHARD, D], BF16, kind="ExternalInput").ap()
    w_d = nc.dram_tensor("w", [128, 512], BF16, kind="ExternalInput").ap()
    o_d = nc.dram_tensor("out", [SHARD, D], BF16, kind="ExternalOutput").ap()

    with tile.TileContext(nc) as tc, ExitStack() as ctx:
        const_pool = ctx.enter_context(tc.tile_pool(name="const", bufs=1))
        xin_pool = ctx.enter_context(tc.tile_pool(name="xin", bufs=cfg["xin_bufs"]))
        out_pool = ctx.enter_context(tc.tile_pool(name="outp", bufs=cfg["out_bufs"]))
        zs_pool = ctx.enter_context(tc.tile_pool(name="zs", bufs=cfg["zs_bufs"]))
        wb_pool = ctx.enter_context(tc.tile_pool(name="wb", bufs=cfg["w_bufs"]))
        ps_zp = ctx.enter_context(
            tc.tile_pool(name="ps_zp", bufs=cfg["z_bufs"], space="PSUM"))

        # All 16 xt loads queued on SP up front; W rides the ACT queue.
        xt_tiles = []
        for it in range(NT):
            xt_sb = xin_pool.tile([128, D], BF16, tag="xt")
            nc.sync.dma_start(xt_sb[:], xt_d[it * 128:(it + 1) * 128, :])
            xt_tiles.append(xt_sb)

        W_sb = const_pool.tile([128, 512], BF16, tag="W")
        nc.scalar.dma_start(W_sb[:], w_d[:])

        # PE p-state warmup: dummy matmuls on a zeroed tile while the first
        # loads are in flight (reusing the z0 PSUM pool).
        Zb_sb = const_pool.tile([128, 256], BF16, tag="Zb")
        nc.vector.memset(Zb_sb[:], 0.0)
        for _ in range(cfg["warmup"]):
            warm_ps = ps_zp.tile([128, 1024], F32, tag="z")
            nc.tensor.matmul(warm_ps[:, 0:256], lhsT=Zb_sb[:, 0:128], rhs=Zb_sb[:],
                             start=True, stop=True)

        for it in range(NT):
            xt_sb = xt_tiles[it]
            last = it == NT - 1
            # all four z quarters share one 2-bank PSUM tile (four
            # accumulation groups): staging to SBUF is ONE 1024-wide ACT copy
            zp = ps_zp.tile([128, 1024], F32, tag="z")
            zs = zs_pool.tile([128, 1024], BF16, tag="zs")
            for a in range(4):
                zps = zp[:, a * 256:(a + 1) * 256]
                for j in range(2):
                    g = 2 * a + j
                    nc.tensor.matmul(
                        zps,
                        lhsT=xt_sb[:, g * 128:(g + 1) * 128],
                        rhs=W_sb[:, j * 256:(j + 1) * 256],
                        start=(j == 0),
                        stop=(j == 1),
                    )
            nc.scalar.copy(zs[:], zp[:])
            zs01 = zs[:, 0:512]
            zs23 = zs[:, 512:1024]

            # H4 butterfly over the a axis, all-SBUF in bf16 (DVE 4x mode,
            # Pool-eligible). zs01 = [z0|z1], zs23 = [z2|z3].
            w0 = wb_pool.tile([128, 256], BF16, tag="w0")
            w1 = wb_pool.tile([128, 256], BF16, tag="w1")
            w2 = wb_pool.tile([128, 256], BF16, tag="w2")
            w3 = wb_pool.tile([128, 256], BF16, tag="w3")
            nc.vector.tensor_add(w0[:], zs01[:, 0:256], zs23[:, 0:256])
            nc.vector.tensor_sub(w2[:], zs01[:, 0:256], zs23[:, 0:256])
            nc.vector.tensor_add(w1[:], zs01[:, 256:512], zs23[:, 256:512])
            nc.vector.tensor_sub(w3[:], zs01[:, 256:512], zs23[:, 256:512])

            ob = out_pool.tile([128, D], BF16, tag="ob")
            row = o_d[it * 128:(it + 1) * 128, :]
            # dist-1 stage: DVE takes y0/y1 (4x bf16 mode), Pool takes y2/y3.
            # One full store per tile on SP: the shared HWDGE generator costs
            # ~625 ns per DMA, so instruction count matters more than shipping
            # halves early (SWDGE would burn ~1 us of Pool ENGINE per store).
            # The final tile keeps everything on DVE and splits its store so
            # the drain tail is as short as possible.
            nc.vector.tensor_add(ob[:, 0:256], w0[:], w1[:])
            nc.vector.tensor_sub(ob[:, 256:512], w0[:], w1[:])
            if last:
                nc.sync.dma_start(row[:, 0:512], ob[:, 0:512])
                nc.vector.tensor_add(ob[:, 512:768], w2[:], w3[:])
                nc.vector.tensor_sub(ob[:, 768:1024], w2[:], w3[:])
                nc.sync.dma_start(row[:, 512:1024], ob[:, 512:1024])
            else:
                nc.gpsimd.tensor_add(ob[:, 512:768], w2[:], w3[:])
                nc.gpsimd.tensor_sub(ob[:, 768:1024], w2[:], w3[:])
                nc.sync.dma_start(row[:], ob[:])

    nc.compile()
    return nc


def _get_nc():
    if "nc" not in _cache:
        _cache["nc"] = _build_nc()
    return _cache["nc"]


def kernel(x, H, **_ignored):
    import ml_dtypes

    x = np.asarray(x, dtype=np.float32)
    H = np.asarray(H, dtype=np.float32)
    nc = _get_nc()

    # Derive the Kronecker factors from the given H (exact when H has the
    # Hadamard structure); fold in the 1/sqrt(1024) scale.
    R = np.ascontiguousarray(H[:128, :128]) * np.float32(1.0 / 32.0)  # symmetric
    H2s = np.ascontiguousarray(H[:2, :2])  # (-1)^popcount(i&j) signs
    # W2[b', j*256 + e8*128 + e_lo] = H2s[e8, j] * R[b', e_lo]
    W = np.ascontiguousarray(
        np.einsum("ej,bl->bjel", H2s, R).reshape(128, 512)
    ).astype(ml_dtypes.bfloat16)

    # Round x to bf16 (the on-chip pipeline would do the same before the
    # 16-bit matmuls) and pre-transpose per 128-row tile:
    # xt[t, b', g, n] = x[t, n, g, b']
    xb = x.reshape(ROWS // 128, 128, 8, 128).astype(ml_dtypes.bfloat16)
    xt = np.ascontiguousarray(xb.transpose(0, 3, 2, 1)).reshape(ROWS, D)

    in_maps = []
    for c in range(N_CORES):
        in_maps.append({
            "xt": np.ascontiguousarray(xt[c * SHARD:(c + 1) * SHARD]),
            "w": W,
        })

    res = bass_utils.run_bass_kernel_spmd(nc, in_maps, core_ids=list(range(N_CORES)))
    y = np.empty((ROWS, D, 2), dtype=np.float32)
    for c in range(N_CORES):
        y[c * SHARD:(c + 1) * SHARD, :, 0] = res.results[c]["out"].astype(np.float32)
    y[:, :, 1] = 0.0
    return y.reshape(B, S, D, 2)
